# revision 6
# baseline (speedup 1.0000x reference)
"""Cost-volume concatenation kernel for Trainium2 (8 NeuronCores).

Reference (per batch b, disparity index d, i = d + MIN_DISP):
  out[b, d, h, w, 0:C]  = left[b, h, w, :]    if 0 <= w - i < W else 0
  out[b, d, h, w, C:2C] = right[b, h, w-i, :] if 0 <= w - i < W else 0

Sharding: disparity-parallel, interleaved -- core c builds disparities
{8j + c : j in 0..15} for the full [B, H, W] volume.  Interleaving
balances valid-span widths (bytes written) across cores.

Precision: the whole on-device datapath runs in bf16 (output values are
exactly the bf16 rounding of the inputs; the host upcasts to f32).  The
rel-err budget is 2e-2 against max|expected| ~ 5; bf16 rounding is
<= 3.9e-3 -- a 5x margin -- and it halves HBM traffic, which is the
roofline for this kernel (~358 GB/s per core).

Layout: the device writes the two channel halves as separate blocks,
out_dev[b, j, h, {0:left | 1:right}, w, c], instead of the interleaved
[w, 2C] rows the caller needs.  The host re-interleaves for free during
the mandatory bf16->f32 upcast pass.  De-interleaving on the device
side makes both halves contiguous per (plane, h) row, so:
  * the right half is stored DIRECTLY from the padded input tile -- the
    disparity shift is a byte offset in the source AP, no compute op;
  * the left half needs a single VectorE multiply per plane (validity
    mask), writing a contiguous [h, span*C] tile the store reads as-is.

SPMD trick: run_bass_kernel_spmd runs ONE program on all 8 cores, so the
per-core offset c cannot appear in any access pattern.  The program is
written for i0 = 8j - 112 and all c-dependence lives in the data:
  * rightp input = right pre-shifted by +c columns, zero-padded to W+8
    columns -- the program's static gather rightp[w - i0] then yields
    right[w - i] with the out-of-range mask applied by the padding.
  * msk input = host-precomputed left-half validity mask over source
    columns w_src = w - i0, channel-expanded:
    msk[w_src*C + ch] = 1.0 iff c <= w_src < W + c.  Host-building it
    (614 KB load, vs 57 MB of stores) removes the on-chip iota/compare
    chain from the first left-store's critical path.
Each plane writes the union-over-c of valid w-spans; columns inside the
union but outside the core's true span receive exact zeros from the
padding/mask; columns outside the union are never written and rely on
ExternalOutput buffers being pre-zeroed (bass2jax donates zero buffers
to PJRT for exactly this purpose).

DMA port balance: the SBUF AXI port swizzle (port = bits[4:2]<<1 |
bit6) maps partitions 0:64 to even ports and 64:128 to odd ports, so a
lone 96-partition stream loads one parity 2:1.  The right-half tiles
live at rows 0:96 (even-heavy) and the left-half tiles at rows 32:128
(odd-heavy); with right stores on the sync HWDGE ring and left stores
on the scalar ring the two concurrent streams cover all 16 ports
evenly -- and each input row is loaded exactly once.

Plane order: within each batch, zipper widest/narrowest so the two
rings see a steady mix of large and small DMAs instead of a cluster of
tiny transfers (with their fixed per-DMA overhead) at the end.
"""

import os
import sys

sys.path.insert(0, "/opt/trn_rl_repo")

import numpy as np
import ml_dtypes

BF16 = ml_dtypes.bfloat16

B, H, W, C = 2, 96, 192, 16
D = 128
MIN_DISP = -112
N_CORES = 8
DPC = D // N_CORES         # 16 disparity planes per core
PAD = 8                    # rightp padded to W + PAD source columns
WP = W + PAD

_CACHE = {}


def _plane_span(j):
    """Union-over-c valid w-span for plane j (program-static)."""
    i0 = 8 * j + MIN_DISP
    if i0 < 0:
        us, ue = 0, min(W + i0 + (N_CORES - 1), W)
    else:
        us, ue = i0, W
    return i0, us, ue


def _plane_order():
    """Zipper: widest, narrowest, 2nd-widest, ... per batch."""
    by_width = sorted(range(DPC), key=lambda j: _plane_span(j)[1] - _plane_span(j)[2])
    order = []
    lo, hi = 0, DPC - 1
    while lo <= hi:
        order.append(by_width[lo])
        if lo != hi:
            order.append(by_width[hi])
        lo += 1
        hi -= 1
    return order


def _build_program():
    from concourse import bacc, mybir
    import concourse.tile as tile

    nc = bacc.Bacc(
        "TRN2", target_bir_lowering=False, debug=False, num_devices=N_CORES
    )
    bf16 = mybir.dt.bfloat16
    left = nc.dram_tensor("left", [B, H, W * C], bf16, kind="ExternalInput")
    rightp = nc.dram_tensor("rightp", [B, H, WP * C], bf16, kind="ExternalInput")
    maskin = nc.dram_tensor("maskin", [96, WP * C], bf16, kind="ExternalInput")
    out = nc.dram_tensor(
        "out", [B, DPC, H, 2, W * C], bf16, kind="ExternalOutput"
    )

    with tile.TileContext(nc) as tc:
        with (
            tc.tile_pool(name="inputs", bufs=1) as ipool,
            tc.tile_pool(name="work", bufs=8) as wpool,
        ):
            # Right-half input tiles at rows 0:96, left at rows 32:128.
            rsb = {b: ipool.tile([128, WP * C], bf16, tag=f"r{b}", name=f"r{b}")
                   for b in range(B)}
            lsb = {b: ipool.tile([128, W * C], bf16, tag=f"l{b}", name=f"l{b}")
                   for b in range(B)}
            msk = ipool.tile([128, WP * C], bf16, tag="msk")

            # b=0 loads at the heads of the two (empty) HWDGE store
            # rings; the mask (first left-mul dependency) leads the
            # SWDGE queue, b=1 tiles behind it.  The mask is only read
            # at rows 32:128 (mul rows 0:32 produce unstored garbage).
            nc.sync.dma_start(rsb[0][0:96, :], rightp.ap()[0])
            nc.scalar.dma_start(lsb[0][32:128, :], left.ap()[0])
            nc.gpsimd.dma_start(msk[32:128, :], maskin.ap())
            for b2 in range(1, B):
                nc.gpsimd.dma_start(rsb[b2][0:96, :], rightp.ap()[b2])
                nc.gpsimd.dma_start(lsb[b2][32:128, :], left.ap()[b2])

            order = _plane_order()
            for b in range(B):
                for j in order:
                    i0, us, ue = _plane_span(j)
                    nw = ue - us
                    x0 = us - i0      # source column offset into rightp/mask

                    # Right half: straight from the padded input tile.
                    nc.sync.dma_start(
                        out.ap()[b, j, :, 1, us * C : ue * C],
                        rsb[b][0:96, x0 * C : (x0 + nw) * C],
                    )

                    # Left half: one masked multiply, then store.
                    # Rows 0:32 of the [0:128) op compute garbage from
                    # never-written input rows and are never stored
                    # (free: op time ~ free size, lanes are parallel).
                    wl = wpool.tile([128, W * C], bf16, tag="wl")
                    nc.vector.tensor_mul(
                        wl[:, us * C : ue * C],
                        lsb[b][:, us * C : ue * C],
                        msk[:, x0 * C : (x0 + nw) * C],
                    )
                    nc.scalar.dma_start(
                        out.ap()[b, j, :, 0, us * C : ue * C],
                        wl[32:128, us * C : ue * C],
                    )

    nc.compile()
    return nc


def _get_program():
    if "nc" not in _CACHE:
        _CACHE["nc"] = _build_program()
    return _CACHE["nc"]


def kernel(left, right):
    from concourse.bass_utils import run_bass_kernel_spmd

    left = np.ascontiguousarray(left, dtype=np.float32)
    right = np.ascontiguousarray(right, dtype=np.float32)
    left_bf = left.astype(BF16).reshape(B, H, W * C)
    right_bf = right.astype(BF16)
    nc = _get_program()

    w_src = np.arange(WP)
    in_maps = []
    for c in range(N_CORES):
        rp = np.zeros((B, H, WP, C), dtype=BF16)
        rp[:, :, c : c + W] = right_bf
        mrow = ((w_src >= c) & (w_src < W + c)).astype(BF16)    # [WP]
        mexp = np.repeat(mrow, C)                                # [WP*C]
        mfull = np.ascontiguousarray(np.broadcast_to(mexp, (96, WP * C)))
        in_maps.append(
            {
                "left": left_bf,
                "rightp": rp.reshape(B, H, WP * C),
                "maskin": mfull,
            }
        )

    prof_dir = os.environ.get("BASS_NTFF_DIR")
    if prof_dir:
        from trn_agent_boot.trn_boot import _ntff_profile_via_ctypes

        hook = _ntff_profile_via_ctypes("/opt/axon/libaxon_pjrt.so")
        with hook(prof_dir, [0]):
            res = run_bass_kernel_spmd(nc, in_maps, core_ids=list(range(N_CORES)))
    else:
        res = run_bass_kernel_spmd(nc, in_maps, core_ids=list(range(N_CORES)))

    # parts[c][b, j] is disparity d = 8j + c; upcast to f32 on the host
    # while re-interleaving the two channel-half blocks.
    full = np.empty((B, DPC, N_CORES, H, W, 2 * C), dtype=np.float32)
    for c in range(N_CORES):
        dev = res.results[c]["out"].reshape(B, DPC, H, 2, W, C)
        full[:, :, c, :, :, 0:C] = dev[:, :, :, 0]
        full[:, :, c, :, :, C : 2 * C] = dev[:, :, :, 1]
    return full.reshape(B, D, H, W, 2 * C)


# revision 7
# speedup vs baseline: 1.0798x; 1.0798x over previous
"""Cost-volume concatenation kernel for Trainium2 (8 NeuronCores).

Reference (per batch b, disparity index d, i = d + MIN_DISP):
  out[b, d, h, w, 0:C]  = left[b, h, w, :]    if 0 <= w - i < W else 0
  out[b, d, h, w, C:2C] = right[b, h, w-i, :] if 0 <= w - i < W else 0

Sharding: disparity-parallel, interleaved -- core c builds disparities
{8j + c : j in 0..15} for the full [B, H, W] volume.  Interleaving
balances valid-span widths (bytes written) across cores.

Precision: the whole on-device datapath runs in bf16 (output values are
exactly the bf16 rounding of the inputs; the host upcasts to f32).  The
rel-err budget is 2e-2 against max|expected| ~ 5; bf16 rounding is
<= 3.9e-3 -- a 5x margin -- and it halves HBM traffic, which is the
roofline for this kernel (~358 GB/s per core).

Layout: the device writes the two channel halves as separate blocks,
out_dev[b, j, h, {0:left | 1:right}, w, c], instead of the interleaved
[w, 2C] rows the caller needs.  The host re-interleaves for free during
the mandatory bf16->f32 upcast pass.  De-interleaving on the device
side makes both halves contiguous per (plane, h) row, so:
  * the right half is stored DIRECTLY from the padded input tile -- the
    disparity shift is a byte offset in the source AP, no compute op;
  * the left half needs a single VectorE multiply per plane (validity
    mask), writing a contiguous [h, span*C] tile the store reads as-is.

SPMD trick: run_bass_kernel_spmd runs ONE program on all 8 cores, so the
per-core offset c cannot appear in any access pattern.  The program is
written for i0 = 8j - 112 and all c-dependence lives in the data:
  * rightp input = right pre-shifted by +c columns, zero-padded to W+8
    columns -- the program's static gather rightp[w - i0] then yields
    right[w - i] with the out-of-range mask applied by the padding.
  * cvec input = per-partition scalars [c, W+c]; the left-half validity
    mask is built on-chip over source columns w_src = w - i0:
    mask[w_src] = (w_src >= c) * (w_src < W+c), materialized as
    [w_src repeated C times] (iota pattern [[1,WP],[0,C]]) so it
    multiplies the channel-expanded rows.  All values are integers
    <= 199, exactly representable in bf16, so the mask pipeline runs in
    bf16 with exact comparisons.  cvec rides at the head of the scalar
    HWDGE ring (~0.7us) so the mask never gates the pipeline.
Each plane writes the union-over-c of valid w-spans; columns inside the
union but outside the core's true span receive exact zeros from the
padding/mask; columns outside the union are never written and rely on
ExternalOutput buffers being pre-zeroed (bass2jax donates zero buffers
to PJRT for exactly this purpose).

DMA port balance: the SBUF AXI port swizzle (port = bits[4:2]<<1 |
bit6) maps partitions 0:64 to even ports and 64:128 to odd ports, so a
lone 96-partition stream loads one parity 2:1.  The right-half tiles
live at rows 0:96 (even-heavy) and the left-half tiles at rows 32:128
(odd-heavy); with right stores on the sync HWDGE ring and left stores
on the scalar ring the two concurrent streams cover all 16 ports
evenly -- and each input row is loaded exactly once.

Scheduling details, all ramp/tail-driven:
  * b=1 input tiles load from INSIDE the store rings (after 8 planes of
    b=0 work) -- an empty SWDGE queue during the first ~10us keeps the
    16 shared SDMA engines free, so the b=0 loads' completion
    semaphores (which gate the first stores) fire promptly.
  * plane order zippers widest/narrowest so both rings see a steady
    mix of large and small DMAs and the tail drains a mid-size plane.
"""

import os
import sys

sys.path.insert(0, "/opt/trn_rl_repo")

import numpy as np
import ml_dtypes

BF16 = ml_dtypes.bfloat16

B, H, W, C = 2, 96, 192, 16
D = 128
MIN_DISP = -112
N_CORES = 8
DPC = D // N_CORES         # 16 disparity planes per core
PAD = 8                    # rightp padded to W + PAD source columns
WP = W + PAD

_CACHE = {}


def _plane_span(j):
    """Union-over-c valid w-span for plane j (program-static)."""
    i0 = 8 * j + MIN_DISP
    if i0 < 0:
        us, ue = 0, min(W + i0 + (N_CORES - 1), W)
    else:
        us, ue = i0, W
    return i0, us, ue


def _plane_order():
    """Zipper: widest, narrowest, 2nd-widest, ... per batch."""
    by_width = sorted(range(DPC), key=lambda j: _plane_span(j)[1] - _plane_span(j)[2])
    order = []
    lo, hi = 0, DPC - 1
    while lo <= hi:
        order.append(by_width[lo])
        if lo != hi:
            order.append(by_width[hi])
        lo += 1
        hi -= 1
    return order


def _build_program():
    from concourse import bacc, mybir
    import concourse.tile as tile

    nc = bacc.Bacc(
        "TRN2", target_bir_lowering=False, debug=False, num_devices=N_CORES
    )
    bf16 = mybir.dt.bfloat16
    f32 = mybir.dt.float32
    left = nc.dram_tensor("left", [B, H, W * C], bf16, kind="ExternalInput")
    rightp = nc.dram_tensor("rightp", [B, H, WP * C], bf16, kind="ExternalInput")
    cvec = nc.dram_tensor("cvec", [128, 2], f32, kind="ExternalInput")
    out = nc.dram_tensor(
        "out", [B, DPC, H, 2, W * C], bf16, kind="ExternalOutput"
    )

    with tile.TileContext(nc) as tc:
        with (
            tc.tile_pool(name="inputs", bufs=1) as ipool,
            tc.tile_pool(name="work", bufs=8) as wpool,
        ):
            # Right-half input tiles at rows 0:96, left at rows 32:128.
            rsb = {b: ipool.tile([128, WP * C], bf16, tag=f"r{b}", name=f"r{b}")
                   for b in range(B)}
            lsb = {b: ipool.tile([128, W * C], bf16, tag=f"l{b}", name=f"l{b}")
                   for b in range(B)}
            cv = ipool.tile([128, 2], f32, tag="cvec")
            msk = ipool.tile([128, WP * C], bf16, tag="msk")
            tmpi = ipool.tile([128, WP * C], bf16, tag="tmpi")

            # b=0 loads at the heads of the two (empty) HWDGE store
            # rings; cvec (1 KB) leads the scalar ring.  iota runs on
            # GpSimd immediately (no deps).  Nothing rides the SWDGE
            # queue this early, so the b=0 loads' completion semaphores
            # are not delayed by packet round-robin on the SDMA engines.
            nc.scalar.dma_start(cv[:, :], cvec.ap())
            nc.sync.dma_start(rsb[0][0:96, :], rightp.ap()[0])
            nc.scalar.dma_start(lsb[0][32:128, :], left.ap()[0])
            nc.gpsimd.iota(
                tmpi[:, :], [[1, WP], [0, C]], channel_multiplier=0,
                allow_small_or_imprecise_dtypes=True,
            )

            # mask over source columns, channel-expanded: 1.0 iff
            # c <= w_src < W + c.  Exact integer compares in bf16.
            nc.vector.tensor_single_scalar(
                msk[:, :], tmpi[:, :], cv[:, 0:1], mybir.AluOpType.is_ge
            )
            nc.vector.tensor_single_scalar(
                tmpi[:, :], tmpi[:, :], cv[:, 1:2], mybir.AluOpType.is_lt
            )
            nc.vector.tensor_mul(msk[:, :], msk[:, :], tmpi[:, :])

            order = _plane_order()
            for b in range(B):
                for k, j in enumerate(order):
                    i0, us, ue = _plane_span(j)
                    nw = ue - us
                    x0 = us - i0      # source column offset into rightp/mask

                    # b=1 tiles load from inside the rings, well after
                    # the ramp but well before b=1 work begins.
                    if b == 0 and k == 8:
                        for b2 in range(1, B):
                            nc.sync.dma_start(rsb[b2][0:96, :], rightp.ap()[b2])
                            nc.scalar.dma_start(lsb[b2][32:128, :], left.ap()[b2])

                    # Right half: straight from the padded input tile.
                    nc.sync.dma_start(
                        out.ap()[b, j, :, 1, us * C : ue * C],
                        rsb[b][0:96, x0 * C : (x0 + nw) * C],
                    )

                    # Left half: one masked multiply, then store.
                    # Rows 0:32 of the [0:128) op compute garbage from
                    # never-written input rows and are never stored
                    # (free: op time ~ free size, lanes are parallel).
                    wl = wpool.tile([128, W * C], bf16, tag="wl")
                    nc.vector.tensor_mul(
                        wl[:, us * C : ue * C],
                        lsb[b][:, us * C : ue * C],
                        msk[:, x0 * C : (x0 + nw) * C],
                    )
                    nc.scalar.dma_start(
                        out.ap()[b, j, :, 0, us * C : ue * C],
                        wl[32:128, us * C : ue * C],
                    )

    nc.compile()
    return nc


def _get_program():
    if "nc" not in _CACHE:
        _CACHE["nc"] = _build_program()
    return _CACHE["nc"]


def kernel(left, right):
    from concourse.bass_utils import run_bass_kernel_spmd

    left = np.ascontiguousarray(left, dtype=np.float32)
    right = np.ascontiguousarray(right, dtype=np.float32)
    left_bf = left.astype(BF16).reshape(B, H, W * C)
    right_bf = right.astype(BF16)
    nc = _get_program()

    in_maps = []
    for c in range(N_CORES):
        rp = np.zeros((B, H, WP, C), dtype=BF16)
        rp[:, :, c : c + W] = right_bf
        cv = np.empty((128, 2), dtype=np.float32)
        cv[:, 0] = float(c)
        cv[:, 1] = float(W + c)
        in_maps.append(
            {
                "left": left_bf,
                "rightp": rp.reshape(B, H, WP * C),
                "cvec": cv,
            }
        )

    prof_dir = os.environ.get("BASS_NTFF_DIR")
    if prof_dir:
        from trn_agent_boot.trn_boot import _ntff_profile_via_ctypes

        hook = _ntff_profile_via_ctypes("/opt/axon/libaxon_pjrt.so")
        with hook(prof_dir, [0]):
            res = run_bass_kernel_spmd(nc, in_maps, core_ids=list(range(N_CORES)))
    else:
        res = run_bass_kernel_spmd(nc, in_maps, core_ids=list(range(N_CORES)))

    # parts[c][b, j] is disparity d = 8j + c; upcast to f32 on the host
    # while re-interleaving the two channel-half blocks.
    full = np.empty((B, DPC, N_CORES, H, W, 2 * C), dtype=np.float32)
    for c in range(N_CORES):
        dev = res.results[c]["out"].reshape(B, DPC, H, 2, W, C)
        full[:, :, c, :, :, 0:C] = dev[:, :, :, 0]
        full[:, :, c, :, :, C : 2 * C] = dev[:, :, :, 1]
    return full.reshape(B, D, H, W, 2 * C)
